# revision 1
# baseline (speedup 1.0000x reference)
"""Trainium2 Bass kernel for nn_Decoder_60232621359478 (dense MoE decoder).

Model (per token): 3-layer gating MLP -> softmax over E=8 experts (dense
weights, all experts active), then 4 MoE layers where each layer is
  y = sum_e ew_e * ([z; x] @ W_e + b_e),  x <- elu(y) (except last layer).

Kernel strategy:
- Data-parallel over batch across 8 NeuronCores (B=32 -> 4 per core,
  1024 tokens/core). No collectives.
- Everything on-chip is FEATURE-MAJOR (features on partitions, tokens on
  the free axis), so layer outputs (PSUM is [Dout, tokens]) feed the next
  layer with no transposes. Host pre-transposes the inputs (free numpy).
- Expert gating folded into the matmul contraction:
    sum_e ew_e * (x @ W_e) = concat_e(ew_e * x) @ stack_e(W_e)
  Scaled inputs (ew_e * x) are produced by DVE right before use; each MoE
  layer is one PSUM-accumulated chain of 8 experts x k-tiles (+ one K=8
  bias matmul with the ew rows as moving operand).
- elu(x)+1 is used as the carried activation (elu+1 = exp(min(x,0)) +
  max(x,0): 2 DVE ops + 1 ACT op); the "-1" is folded into the next
  layer's bias on the host (b' = b - colsum(W_xpart)).
- All matmuls run in float32r (fp32 storage, ~1 cycle/row on the PE vs 4
  for strict fp32; rel err ~1.5e-4).
- Softmax over the 8 experts (partition axis) is done with PE tricks:
  colsum via ones(8,1) matmul, row-broadcasts via one-hot stationary
  matmuls, reciprocal on DVE.
"""

import numpy as np

import concourse.bass as bass
import concourse.mybir as mybir
import concourse.tile as tile
from concourse import bacc
from concourse import bass_utils

dt = mybir.dt
AF = mybir.ActivationFunctionType
ALU = mybir.AluOpType

B, T = 32, 256
DM, DL, DH, DP, E = 256, 256, 512, 16, 8
NCORES = 8
BP = B // NCORES            # batches per core
NT = BP * T                 # tokens per core (1024)
CH = 2                      # token chunks
CT = NT // CH               # tokens per chunk (512)

_CACHE = {}


def _prep_weights(gw0, gb0, gw1, gb1, gw2, gb2,
                  w0, b0, w1, b1, w2, b2, wo, bo):
    f = np.float32
    # gating: k-tiles [z0, z1, extra]; extra rows 0:16 = p-part, row 16 = bias
    G0 = np.zeros((3, 128, DH), f)
    G0[0] = gw0[0:128]
    G0[1] = gw0[128:256]
    G0[2, 0:16] = gw0[256:272]
    G0[2, 16] = gb0

    def g_later(gw, gb, dout):
        Gt = np.zeros((7, 128, dout), f)
        Gt[0:6] = gw[0:768].reshape(6, 128, dout)
        Gt[6, 16] = gb - gw[256:768].sum(axis=0)   # h' = elu+1 correction
        return Gt

    G1 = g_later(gw1, gb1, DH)
    G2 = g_later(gw2, gb2, E)

    # L0: input tiles [z0, z1, xc0, xc1, (v,pad)]; w0 rows are [z, v, xc]
    W0 = np.zeros((E, 5, 128, DH), f)
    W0[:, 0] = w0[:, 0:128]
    W0[:, 1] = w0[:, 128:256]
    W0[:, 2] = w0[:, 259:387]
    W0[:, 3] = w0[:, 387:515]
    W0[:, 4, 0:3] = w0[:, 256:259]
    B0 = b0.astype(f)

    def moe_later(w, b):
        Wt = np.ascontiguousarray(w.reshape(E, 6, 128, -1).astype(f))
        Bt = (b - w[:, 256:768, :].sum(axis=1)).astype(f)
        return Wt, Bt

    W1, B1 = moe_later(w1, b1)
    W2, B2 = moe_later(w2, b2)
    WO, BO = moe_later(wo, bo)

    ONES = np.ones((E, 128), f)
    EMAT = np.zeros((E, E * 128), f)
    for e in range(E):
        EMAT[e, e * 128:(e + 1) * 128] = 1.0
    return dict(G0=G0, G1=G1, G2=G2, W0=W0, B0=B0, W1=W1, B1=B1,
                W2=W2, B2=B2, WO=WO, BO=BO, ONES=ONES, EMAT=EMAT)


def _prep_core_inputs(z, p_next, v_hip_next, x_curr, core):
    f = np.float32
    sl = slice(core * BP, (core + 1) * BP)
    zT = np.ascontiguousarray(z[sl].reshape(NT, DL).T.astype(f))
    x0T = np.zeros((384, NT), f)
    x0T[0:256] = x_curr[sl].reshape(NT, DM).T
    x0T[256:259] = v_hip_next[sl].reshape(NT, 3).T
    gex = np.zeros((128, NT), f)
    gex[0:16] = p_next[sl].reshape(NT, DP).T
    gex[16] = 1.0
    return dict(zT=zT, x0T=x0T, gex=gex)


def _build(repeat=1, scope="all"):
    nc = bacc.Bacc("TRN2", target_bir_lowering=False, debug=False,
                   num_devices=NCORES)
    fr = dt.float32r

    def din(name, shape):
        return nc.dram_tensor(name, shape, fr, kind="ExternalInput").ap()

    zT_d = din("zT", (DL, NT))
    x0T_d = din("x0T", (384, NT))
    gex_d = din("gex", (128, NT))
    ones_d = din("ONES", (E, 128))
    emat_d = din("EMAT", (E, E * 128))
    G0_d = din("G0", (3, 128, DH))
    G1_d = din("G1", (7, 128, DH))
    G2_d = din("G2", (7, 128, E))
    W0_d = din("W0", (E, 5, 128, DH))
    W1_d = din("W1", (E, 6, 128, DH))
    W2_d = din("W2", (E, 6, 128, DH))
    WO_d = din("WO", (E, 6, 128, DM))
    B0_d = din("B0", (E, DH))
    B1_d = din("B1", (E, DH))
    B2_d = din("B2", (E, DH))
    BO_d = din("BO", (E, DM))
    yT_d = nc.dram_tensor("yT", (DM, NT), dt.float32,
                          kind="ExternalOutput").ap()

    with tile.TileContext(nc) as tc, \
         nc.allow_low_precision(reason="float32r matmul rounding intended"):
        with tc.tile_pool(name="inp", bufs=1) as inp, \
             tc.tile_pool(name="wp", bufs=3) as wp, \
             tc.tile_pool(name="act", bufs=1) as act, \
             tc.tile_pool(name="xsp", bufs=8) as xsp, \
             tc.tile_pool(name="tmp", bufs=4) as tmpp, \
             tc.tile_pool(name="ps", bufs=8, space="PSUM") as ps:

            # ---- persistent inputs ----
            z_sb = inp.tile([128, 2, NT], fr, name="z_sb")
            nc.sync.dma_start(z_sb, zT_d.rearrange("(k p) t -> p k t", p=128))
            x0_sb = inp.tile([128, 3, NT], fr, name="x0_sb")
            nc.sync.dma_start(x0_sb, x0T_d.rearrange("(k p) t -> p k t", p=128))
            gex_sb = inp.tile([128, NT], fr, name="gex_sb")
            nc.sync.dma_start(gex_sb, gex_d)
            ones_sb = inp.tile([E, 128], fr, name="ones_sb")
            nc.sync.dma_start(ones_sb, ones_d)
            emat_sb = inp.tile([E, E * 128], fr, name="emat_sb")
            nc.sync.dma_start(emat_sb, emat_d)
            g0_sb = inp.tile([128, 3, DH], fr, name="g0_sb")
            nc.sync.dma_start(g0_sb, G0_d.rearrange("k p d -> p k d"))
            g1_sb = inp.tile([128, 7, DH], fr, name="g1_sb")
            nc.sync.dma_start(g1_sb, G1_d.rearrange("k p d -> p k d"))
            g2_sb = inp.tile([128, 7, E], fr, name="g2_sb")
            nc.sync.dma_start(g2_sb, G2_d.rearrange("k p d -> p k d"))
            bias_sb = []
            for i, (bd, dout) in enumerate(
                    [(B0_d, DH), (B1_d, DH), (B2_d, DH), (BO_d, DM)]):
                bt = inp.tile([E, dout], fr, name=f"b{i}_sb")
                nc.sync.dma_start(bt, bd)
                bias_sb.append(bt)

            def elu_p1(dst, psum):
                """dst = elu(psum) + 1 = exp(min(psum,0)) + max(psum,0).

                min(x,0) = -relu(-x); both unary steps run on ACT so the
                DVE (busy producing scaled inputs) only pays one op.
                """
                mn = tmpp.tile([psum.shape[0], psum.shape[-1]], dt.float32,
                               name="mn", tag="mn")
                nc.scalar.activation(mn[:, :], psum, AF.Relu, scale=-1.0)
                ex = tmpp.tile([psum.shape[0], psum.shape[-1]], dt.float32,
                               name="ex", tag="ex")
                nc.scalar.activation(ex[:, :], mn[:, :], AF.Exp, scale=-1.0)
                nc.vector.scalar_tensor_tensor(
                    dst, psum, 0.0, ex[:, :], ALU.max, ALU.add)

            def body_gate():
                # ---- gating MLP ----
                def glayer(w_sb, ktiles, rhs_of, douts, dst_of,
                           kt_order=None):
                    psums = [[ps.tile([128, CT], dt.float32,
                                      name=f"gps{m}_{c}", tag="ps")
                              for c in range(CH)] for m in range(douts)]
                    order = list(kt_order) if kt_order else list(range(ktiles))
                    for kt in order:
                        for c in range(CH):
                            cs = slice(c * CT, (c + 1) * CT)
                            rhs = rhs_of(kt, cs)
                            for m in range(douts):
                                nc.tensor.matmul(
                                    psums[m][c][:, :] if douts > 1
                                    else psums[m][c][:E, :],
                                    w_sb[:, kt, m * 128:(m + 1) * 128]
                                    if douts > 1 else w_sb[:, kt, :],
                                    rhs,
                                    start=(kt == order[0]),
                                    stop=(kt == order[-1]))
                    for c in range(CH):
                        cs = slice(c * CT, (c + 1) * CT)
                        dst_of(c, cs, [psums[m][c] for m in range(douts)])

                h0 = [act.tile([128, NT], fr, name=f"h0_{m}", tag="xp", bufs=8)
                      for m in range(4)]

                def rhs_g0(kt, cs):
                    return (z_sb[:, kt, cs] if kt < 2 else gex_sb[:, cs])

                def dst_h0(c, cs, psums):
                    for m in range(4):
                        elu_p1(h0[m][:, cs], psums[m][:, :])

                glayer(g0_sb, 3, rhs_g0, 4, dst_h0)

                h1 = [act.tile([128, NT], fr, name=f"h1_{m}", tag="xp", bufs=8)
                      for m in range(4)]

                def rhs_g1(kt, cs):
                    if kt < 2:
                        return z_sb[:, kt, cs]
                    if kt < 6:
                        return h0[kt - 2][:, cs]
                    return gex_sb[:, cs]

                def dst_h1(c, cs, psums):
                    for m in range(4):
                        elu_p1(h1[m][:, cs], psums[m][:, :])

                glayer(g1_sb, 7, rhs_g1, 4, dst_h1,
                       kt_order=[0, 1, 6, 2, 3, 4, 5])

                exp_g = act.tile([E, NT], fr, name="exp_g", tag="eg")

                def rhs_g2(kt, cs):
                    if kt < 2:
                        return z_sb[:, kt, cs]
                    if kt < 6:
                        return h1[kt - 2][:, cs]
                    return gex_sb[:, cs]

                def dst_g2(c, cs, psums):
                    nc.scalar.activation(exp_g[:, cs], psums[0][:E, :], AF.Exp)

                glayer(g2_sb, 7, rhs_g2, 1, dst_g2,
                       kt_order=[0, 1, 6, 2, 3, 4, 5])

                # ---- softmax normalization (partition axis, via PE) ----
                recip = act.tile([1, NT], fr, name="recip", tag="rc")
                rbc = act.tile([128, NT], dt.float32, name="rbc", tag="rbc")
                for c in range(CH):
                    cs = slice(c * CT, (c + 1) * CT)
                    s_ps = ps.tile([1, CT], dt.float32, name="s_ps", tag="ps")
                    nc.tensor.matmul(s_ps[:, :], ones_sb[:, 0:1], exp_g[:, cs],
                                     start=True, stop=True)
                    nc.vector.reciprocal(recip[:, cs], s_ps[:, :])
                    rb_ps = ps.tile([128, CT], dt.float32, name="rb_ps", tag="ps")
                    nc.tensor.matmul(rb_ps[:, :], ones_sb[0:1, :], recip[:, cs],
                                     start=True, stop=True)
                    nc.scalar.copy(rbc[:, cs], rb_ps[:, :])

                ew8 = act.tile([E, NT], fr, name="ew8", tag="ew8")
                nc.vector.tensor_mul(ew8[:, :], exp_g[:, :], rbc[:E, :])
                ewb = [act.tile([128, NT], fr, name=f"ewb{e}", tag="ewb", bufs=8)
                       for e in range(E)]
                for e in range(E):
                    for c in range(CH):
                        cs = slice(c * CT, (c + 1) * CT)
                        eb_ps = ps.tile([128, CT], dt.float32,
                                        name="eb_ps", tag="ps")
                        nc.tensor.matmul(
                            eb_ps[:, :], emat_sb[:, e * 128:(e + 1) * 128],
                            exp_g[:, cs], start=True, stop=True)
                        nc.vector.tensor_mul(ewb[e][:, cs], eb_ps[:, :],
                                             rbc[:, cs])

                return ew8, ewb, h0

            def body_moe(gate_out):
                ew8, ewb = gate_out[0], gate_out[1]
                # ---- MoE layers ----
                layers = [
                    (W0_d, bias_sb[0], 5, 4, DH),
                    (W1_d, bias_sb[1], 6, 4, DH),
                    (W2_d, bias_sb[2], 6, 4, DH),
                    (WO_d, bias_sb[3], 6, 2, DM),
                ]
                xcur = None   # list of 4 act tiles (128, NT) for layers >= 1
                y_sb = None

                for li, (wd, b_sb, ktiles, douts, dout_dim) in enumerate(layers):
                    w_tiles = []
                    for e in range(E):
                        wt = wp.tile([128, ktiles, dout_dim], fr,
                                     name=f"w{li}_{e}", tag="w", bufs=3)
                        nc.sync.dma_start(wt, wd[e].rearrange("k p d -> p k d"))
                        w_tiles.append(wt)

                    if li < 3:
                        xnext = [act.tile([128, NT], fr, name=f"x{li + 1}_{m}",
                                          tag="xp", bufs=8) for m in range(4)]
                    else:
                        y_sb = [act.tile([128, NT], dt.float32, name=f"y{m}",
                                         tag="xp", bufs=8) for m in range(2)]

                    def xsrc(kt, cs):
                        if kt < 2:
                            return z_sb[:, kt, cs]
                        if li == 0:
                            return x0_sb[:, kt - 2, cs]
                        return xcur[kt - 2][:, cs]

                    psums = [[ps.tile([128, CT], dt.float32,
                                      name=f"mps{li}_{m}_{c}", tag="ps")
                              for c in range(CH)] for m in range(douts)]
                    for c in range(CH):
                        cs = slice(c * CT, (c + 1) * CT)
                        for m in range(douts):
                            nc.tensor.matmul(
                                psums[m][c][:, :], b_sb[:, m * 128:(m + 1) * 128],
                                ew8[:, cs], start=True, stop=False)
                    for e in range(E):
                        for kt in range(ktiles):
                            for c in range(CH):
                                cs = slice(c * CT, (c + 1) * CT)
                                xs = xsp.tile([128, CT], fr, name="xs", tag="xs")
                                nc.vector.tensor_mul(xs[:, :], xsrc(kt, cs),
                                                     ewb[e][:, cs])
                                for m in range(douts):
                                    nc.tensor.matmul(
                                        psums[m][c][:, :],
                                        w_tiles[e][:, kt, m * 128:(m + 1) * 128],
                                        xs[:, :],
                                        start=False,
                                        stop=(e == E - 1 and kt == ktiles - 1))
                    for c in range(CH):
                        cs = slice(c * CT, (c + 1) * CT)
                        if li < 3:
                            for m in range(douts):
                                elu_p1(xnext[m][:, cs], psums[m][c][:, :])
                        else:
                            for m in range(douts):
                                nc.scalar.copy(y_sb[m][:, cs], psums[m][c][:, :])
                    if li < 3:
                        xcur = xnext

                for m in range(2):
                    nc.sync.dma_start(yT_d[m * 128:(m + 1) * 128, :], y_sb[m])

            HINTS = (mybir.EngineType.PE, mybir.EngineType.DVE,
                     mybir.EngineType.Activation, mybir.EngineType.SP)
            if repeat == 1:
                body_moe(body_gate())
            elif scope == "all":
                with tc.For_i(0, repeat, 1, hint_engines=HINTS):
                    body_moe(body_gate())
            elif scope == "gating":
                with tc.For_i(0, repeat, 1, hint_engines=HINTS):
                    body_gate()
                body_moe(body_gate())
            elif scope == "moe":
                gout = body_gate()
                with tc.For_i(0, repeat, 1, hint_engines=HINTS):
                    body_moe(gout)
            elif scope == "moe1":
                ew8, ewb, h0 = body_gate()
                with tc.For_i(0, repeat, 1, hint_engines=HINTS):
                    # one L1-shaped layer: weights DMA'd, src = h0 tiles
                    wt1 = []
                    for e in range(E):
                        wt = wp.tile([128, 6, DH], fr,
                                     name=f"m1w_{e}", tag="w", bufs=3)
                        nc.sync.dma_start(wt, W1_d[e].rearrange(
                            "k p d -> p k d"))
                        wt1.append(wt)
                    xnext1 = [act.tile([128, NT], fr, name=f"m1x_{m}",
                                       tag="xp", bufs=8) for m in range(4)]
                    psums1 = [[ps.tile([128, CT], dt.float32,
                                       name=f"m1ps_{m}_{c}", tag="ps")
                               for c in range(CH)] for m in range(4)]
                    for c in range(CH):
                        cs = slice(c * CT, (c + 1) * CT)
                        for m in range(4):
                            nc.tensor.matmul(
                                psums1[m][c][:, :],
                                bias_sb[1][:, m * 128:(m + 1) * 128],
                                ew8[:, cs], start=True, stop=False)
                    for e in range(E):
                        for kt in range(6):
                            for c in range(CH):
                                cs = slice(c * CT, (c + 1) * CT)
                                src_ap = (z_sb[:, kt, cs] if kt < 2
                                          else h0[kt - 2][:, cs])
                                xs = xsp.tile([128, CT], fr, name="xs",
                                              tag="xs")
                                nc.vector.tensor_mul(xs[:, :], src_ap,
                                                     ewb[e][:, cs])
                                for m in range(4):
                                    nc.tensor.matmul(
                                        psums1[m][c][:, :],
                                        wt1[e][:, kt, m * 128:(m + 1) * 128],
                                        xs[:, :], start=False,
                                        stop=(e == E - 1 and kt == 5))
                    for c in range(CH):
                        cs = slice(c * CT, (c + 1) * CT)
                        for m in range(4):
                            elu_p1(xnext1[m][:, cs], psums1[m][c][:, :])
                body_moe((ew8, ewb))
            else:
                raise ValueError(scope)


    nc.compile()
    return nc


def kernel(z, p_next, v_hip_next, x_curr,
           gw0, gb0, gw1, gb1, gw2, gb2,
           w0, b0, w1, b1, w2, b2, wo, bo):
    if "nc" not in _CACHE:
        _CACHE["nc"] = _build()
    nc = _CACHE["nc"]

    wdict = _prep_weights(
        np.asarray(gw0, np.float32), np.asarray(gb0, np.float32),
        np.asarray(gw1, np.float32), np.asarray(gb1, np.float32),
        np.asarray(gw2, np.float32), np.asarray(gb2, np.float32),
        np.asarray(w0, np.float32), np.asarray(b0, np.float32),
        np.asarray(w1, np.float32), np.asarray(b1, np.float32),
        np.asarray(w2, np.float32), np.asarray(b2, np.float32),
        np.asarray(wo, np.float32), np.asarray(bo, np.float32))

    in_maps = []
    for c in range(NCORES):
        m = _prep_core_inputs(np.asarray(z, np.float32),
                              np.asarray(p_next, np.float32),
                              np.asarray(v_hip_next, np.float32),
                              np.asarray(x_curr, np.float32), c)
        m.update(wdict)
        in_maps.append(m)

    res = bass_utils.run_bass_kernel_spmd(
        nc, in_maps, core_ids=list(range(NCORES)))

    out = np.empty((B, T, DM), np.float32)
    for c in range(NCORES):
        yT = res.results[c]["yT"]                     # (DM, NT)
        out[c * BP:(c + 1) * BP] = yT.T.reshape(BP, T, DM)
    return out



# revision 3
# speedup vs baseline: 14001.1811x; 14001.1811x over previous
"""Trainium2 Bass kernel for nn_Decoder_60232621359478 (dense MoE decoder).

Model (per token): 3-layer gating MLP -> softmax over E=8 experts (dense
weights, all experts active), then 4 MoE layers where each layer is
  y = sum_e ew_e * ([z; x] @ W_e + b_e),  x <- elu(y) (except last layer).

Kernel strategy:
- Data-parallel over batch across 8 NeuronCores (B=32 -> 4 per core,
  1024 tokens/core). No collectives.
- Everything on-chip is FEATURE-MAJOR (features on partitions, tokens on
  the free axis), so layer outputs (PSUM is [Dout, tokens]) feed the next
  layer with no transposes. Host pre-transposes inputs AND weights
  (weight tiles land in [partition, ktile, dout] order so every weight
  DMA is a single contiguous read).
- bf16 datapath: weights, activations and the per-expert scaled inputs
  are bf16 (PSUM accumulation stays fp32).  This gives the DVE its
  2x_1P mode (the 368 per-expert scaling multiplies are the #2 engine
  load), halves HBM weight traffic and halves host->device bytes.
  The carried activation is elu(y) itself (NOT elu+1): bf16 rounding is
  relative, so carrying the small-magnitude value keeps the absolute
  injected noise ~2x smaller (sim: 5.2e-3 vs 1.1e-2 max-rel error).
- Expert gating folded into the matmul contraction:
    sum_e ew_e * (x @ W_e) = concat_e(ew_e * x) @ stack_e(W_e)
  Scaled inputs (ew_e * x) are produced by DVE right before use; each MoE
  layer is one PSUM-accumulated chain of 8 experts x k-tiles plus one
  "bias" matmul whose moving operand carries the softmax weights.
- Layer-0's 3 v_hip rows are folded into that bias matmul (moving rows
  [ew8(8); v*ew_e(24)]) instead of wasting a 128-row k-tile per expert:
  -64 PE matmuls.  The 32-row moving operand is built with two tiny
  matmuls (a constant row-scatter of [v;1] and of ew8) + one DVE mul.
- elu(y) = (exp(min(y,0)) - 1) + max(y,0) as 3 ACT ops + 1 DVE op:
    mn = Relu(-y); ex = Exp(-mn); pos = Relu(y)
    x  = (ex + (-1)) + pos        (scalar_tensor_tensor)
- Softmax over the 8 experts (partition axis) via PE tricks: colsum via
  ones(8,1) matmul, 1->8 and 8->128 row-broadcasts via one-hot
  stationary matmuls, reciprocal on DVE.
"""

import numpy as np
import ml_dtypes

import concourse.bass as bass
import concourse.mybir as mybir
import concourse.tile as tile
from concourse import bacc
from concourse import bass_utils

dt = mybir.dt
AF = mybir.ActivationFunctionType
ALU = mybir.AluOpType
BF = ml_dtypes.bfloat16

B, T = 32, 256
DM, DL, DH, DP, E = 256, 256, 512, 16, 8
NCORES = 8
BP = B // NCORES            # batches per core
NT = BP * T                 # tokens per core (1024)
CH = 2                      # token chunks
CT = NT // CH               # tokens per chunk (512)

_CACHE = {}


def _prep_weights(gw0, gb0, gw1, gb1, gw2, gb2,
                  w0, b0, w1, b1, w2, b2, wo, bo):
    f = np.float32

    def pk(a):  # [kt, 128, d] -> [128, kt, d] contiguous bf16
        return np.ascontiguousarray(a.transpose(1, 0, 2)).astype(BF)

    # gating k-tiles: [z0, z1, extra]; extra rows 0:16 = p-part, row 16 = bias
    G0 = np.zeros((3, 128, DH), f)
    G0[0] = gw0[0:128]
    G0[1] = gw0[128:256]
    G0[2, 0:16] = gw0[256:272]
    G0[2, 16] = gb0

    def g_later(gw, gb, dout):
        Gt = np.zeros((7, 128, dout), f)
        Gt[0:6] = gw[0:768].reshape(6, 128, dout)
        Gt[6, 16] = gb
        return Gt

    G1 = g_later(gw1, gb1, DH)
    G2 = g_later(gw2, gb2, E)

    # L0: k-tiles [z0, z1, xc0, xc1]; w0 rows are [z(256), v(3), xc(256)]
    W0 = np.zeros((E, 4, 128, DH), f)
    W0[:, 0] = w0[:, 0:128]
    W0[:, 1] = w0[:, 128:256]
    W0[:, 2] = w0[:, 259:387]
    W0[:, 3] = w0[:, 387:515]

    def pk4(a):  # [E, kt, 128, d] -> [E, 128, kt, d] contiguous bf16
        return np.ascontiguousarray(a.transpose(0, 2, 1, 3)).astype(BF)

    W1 = w1.reshape(E, 6, 128, DH)
    W2 = w2.reshape(E, 6, 128, DH)
    WO = wo.reshape(E, 6, 128, DM)

    # L0 bias+v stationary: rows 0:8 = b0[e], row 8+3e+j = w0[e, 256+j]
    BV = np.zeros((32, DH), f)
    BV[0:8] = b0
    for e in range(E):
        for j in range(3):
            BV[8 + 3 * e + j] = w0[e, 256 + j]

    ONES = np.ones((E, 128), f)
    EMAT = np.zeros((E, E * 128), f)
    for e in range(E):
        EMAT[e, e * 128:(e + 1) * 128] = 1.0
    # VP: scatter [v(3 rows at p0:3); 1.0 at p4] -> 32 rows
    VP = np.zeros((128, 32), f)
    VP[4, 0:8] = 1.0
    for e in range(E):
        for j in range(3):
            VP[j, 8 + 3 * e + j] = 1.0
    # EM2: scatter ew8 -> [ew(8); ew_e at rows 8+3e+j]
    EM2 = np.zeros((E, 32), f)
    for e in range(E):
        EM2[e, e] = 1.0
        for j in range(3):
            EM2[e, 8 + 3 * e + j] = 1.0

    return dict(G0=pk(G0), G1=pk(G1), G2=pk(G2),
                W0=pk4(W0), W1=pk4(W1), W2=pk4(W2), WO=pk4(WO),
                BV=BV.astype(BF), B1=b1.astype(BF), B2=b2.astype(BF),
                BO=bo.astype(BF), ONES=ONES.astype(BF),
                EMAT=EMAT.astype(BF), VP=VP.astype(BF), EM2=EM2.astype(BF))


def _prep_core_inputs(z, p_next, v_hip_next, x_curr, core):
    f = np.float32
    sl = slice(core * BP, (core + 1) * BP)
    zT = z[sl].reshape(NT, DL).T                     # (256, NT)
    zT2 = np.ascontiguousarray(
        zT.reshape(2, 128, NT).transpose(1, 0, 2)).astype(BF)
    x0T = np.zeros((128, 3, NT), f)
    xcT = x_curr[sl].reshape(NT, DM).T               # (256, NT)
    x0T[:, 0] = xcT[0:128]
    x0T[:, 1] = xcT[128:256]
    x0T[0:3, 2] = v_hip_next[sl].reshape(NT, 3).T
    x0T[4, 2] = 1.0
    gex = np.zeros((128, NT), f)
    gex[0:16] = p_next[sl].reshape(NT, DP).T
    gex[16] = 1.0
    return dict(zT=zT2, x0T=x0T.astype(BF), gex=gex.astype(BF))


def _build(repeat=1, scope="all"):
    nc = bacc.Bacc("TRN2", target_bir_lowering=False, debug=False,
                   num_devices=NCORES)
    bf = dt.bfloat16

    def din(name, shape):
        return nc.dram_tensor(name, shape, bf, kind="ExternalInput").ap()

    zT_d = din("zT", (128, 2, NT))
    x0T_d = din("x0T", (128, 3, NT))
    gex_d = din("gex", (128, NT))
    ones_d = din("ONES", (E, 128))
    emat_d = din("EMAT", (E, E * 128))
    vp_d = din("VP", (128, 32))
    em2_d = din("EM2", (E, 32))
    G0_d = din("G0", (128, 3, DH))
    G1_d = din("G1", (128, 7, DH))
    G2_d = din("G2", (128, 7, E))
    W0_d = din("W0", (E, 128, 4, DH))
    W1_d = din("W1", (E, 128, 6, DH))
    W2_d = din("W2", (E, 128, 6, DH))
    WO_d = din("WO", (E, 128, 6, DM))
    BV_d = din("BV", (32, DH))
    B1_d = din("B1", (E, DH))
    B2_d = din("B2", (E, DH))
    BO_d = din("BO", (E, DM))
    yT_d = nc.dram_tensor("yT", (DM, NT), bf, kind="ExternalOutput").ap()

    with tile.TileContext(nc) as tc, \
         nc.allow_low_precision(reason="bf16 datapath intended"):
        with tc.tile_pool(name="inp", bufs=1) as inp, \
             tc.tile_pool(name="wp", bufs=3) as wp, \
             tc.tile_pool(name="act", bufs=1) as act, \
             tc.tile_pool(name="xsp", bufs=8) as xsp, \
             tc.tile_pool(name="tmp", bufs=4) as tmpp, \
             tc.tile_pool(name="ps", bufs=8, space="PSUM") as ps:

            # ---- persistent inputs ----
            z_sb = inp.tile([128, 2, NT], bf, name="z_sb")
            nc.sync.dma_start(z_sb, zT_d)
            x0_sb = inp.tile([128, 3, NT], bf, name="x0_sb")
            nc.sync.dma_start(x0_sb, x0T_d)
            gex_sb = inp.tile([128, NT], bf, name="gex_sb")
            nc.sync.dma_start(gex_sb, gex_d)
            ones_sb = inp.tile([E, 128], bf, name="ones_sb")
            nc.sync.dma_start(ones_sb, ones_d)
            emat_sb = inp.tile([E, E * 128], bf, name="emat_sb")
            nc.sync.dma_start(emat_sb, emat_d)
            vp_sb = inp.tile([128, 32], bf, name="vp_sb")
            nc.sync.dma_start(vp_sb, vp_d)
            em2_sb = inp.tile([E, 32], bf, name="em2_sb")
            nc.sync.dma_start(em2_sb, em2_d)
            g0_sb = inp.tile([128, 3, DH], bf, name="g0_sb")
            nc.sync.dma_start(g0_sb, G0_d)
            g1_sb = inp.tile([128, 7, DH], bf, name="g1_sb")
            nc.sync.dma_start(g1_sb, G1_d)
            g2_sb = inp.tile([128, 7, E], bf, name="g2_sb")
            nc.sync.dma_start(g2_sb, G2_d)
            bv_w = inp.tile([32, DH], bf, name="bv_w")
            nc.sync.dma_start(bv_w, BV_d)
            bias_sb = [None]
            for i, (bd, dout) in enumerate(
                    [(B1_d, DH), (B2_d, DH), (BO_d, DM)]):
                bt = inp.tile([E, dout], bf, name=f"b{i + 1}_sb")
                nc.sync.dma_start(bt, bd)
                bias_sb.append(bt)

            def elu(dst, psum):
                """dst = elu(psum) = (exp(min(psum,0)) - 1) + max(psum,0).

                min(x,0) = -relu(-x); the three unary steps run on ACT so
                the DVE (busy producing scaled inputs) only pays one op.
                """
                mn = tmpp.tile([psum.shape[0], psum.shape[-1]], bf,
                               name="mn", tag="mn")
                nc.scalar.activation(mn[:, :], psum, AF.Relu, scale=-1.0)
                ex = tmpp.tile([psum.shape[0], psum.shape[-1]], bf,
                               name="ex", tag="ex")
                nc.scalar.activation(ex[:, :], mn[:, :], AF.Exp, scale=-1.0)
                pos = tmpp.tile([psum.shape[0], psum.shape[-1]], bf,
                                name="pos", tag="pos")
                nc.scalar.activation(pos[:, :], psum, AF.Relu)
                nc.vector.scalar_tensor_tensor(
                    dst, ex[:, :], -1.0, pos[:, :], ALU.add, ALU.add)

            def body_gate():
                # ---- gating MLP ----
                def glayer(w_sb, ktiles, rhs_of, douts, dst_of,
                           kt_order=None):
                    psums = [[ps.tile([128, CT], dt.float32,
                                      name=f"gps{m}_{c}", tag="ps")
                              for c in range(CH)] for m in range(douts)]
                    order = list(kt_order) if kt_order else list(range(ktiles))
                    for kt in order:
                        for c in range(CH):
                            cs = slice(c * CT, (c + 1) * CT)
                            rhs = rhs_of(kt, cs)
                            for m in range(douts):
                                nc.tensor.matmul(
                                    psums[m][c][:, :] if douts > 1
                                    else psums[m][c][:E, :],
                                    w_sb[:, kt, m * 128:(m + 1) * 128]
                                    if douts > 1 else w_sb[:, kt, :],
                                    rhs,
                                    start=(kt == order[0]),
                                    stop=(kt == order[-1]))
                    dst_of(psums)

                h0 = [act.tile([128, NT], bf, name=f"h0_{m}",
                               tag="xp", bufs=8) for m in range(4)]

                def rhs_g0(kt, cs):
                    return (z_sb[:, kt, cs] if kt < 2 else gex_sb[:, cs])

                def dst_h0(psums):
                    for m in range(4):
                        for c in range(CH):
                            cs = slice(c * CT, (c + 1) * CT)
                            elu(h0[m][:, cs], psums[m][c][:, :])

                glayer(g0_sb, 3, rhs_g0, 4, dst_h0)

                h1 = [act.tile([128, NT], bf, name=f"h1_{m}",
                               tag="xp", bufs=8) for m in range(4)]

                def rhs_g1(kt, cs):
                    if kt < 2:
                        return z_sb[:, kt, cs]
                    if kt < 6:
                        return h0[kt - 2][:, cs]
                    return gex_sb[:, cs]

                def dst_h1(psums):
                    for m in range(4):
                        for c in range(CH):
                            cs = slice(c * CT, (c + 1) * CT)
                            elu(h1[m][:, cs], psums[m][c][:, :])

                glayer(g1_sb, 7, rhs_g1, 4, dst_h1,
                       kt_order=[0, 1, 6, 2, 3, 4, 5])

                exp_g = act.tile([E, NT], bf, name="exp_g", tag="eg")

                def rhs_g2(kt, cs):
                    if kt < 2:
                        return z_sb[:, kt, cs]
                    if kt < 6:
                        return h1[kt - 2][:, cs]
                    return gex_sb[:, cs]

                def dst_g2(psums):
                    for c in range(CH):
                        cs = slice(c * CT, (c + 1) * CT)
                        nc.scalar.activation(exp_g[:, cs],
                                             psums[0][c][:E, :], AF.Exp)

                glayer(g2_sb, 7, rhs_g2, 1, dst_g2,
                       kt_order=[0, 1, 6, 2, 3, 4, 5])

                # ---- softmax normalization (partition axis, via PE) ----
                recip = act.tile([1, NT], bf, name="recip", tag="rc")
                rb8 = act.tile([E, NT], bf, name="rb8", tag="rb8")
                ew8 = act.tile([E, NT], bf, name="ew8", tag="ew8")
                for c in range(CH):
                    cs = slice(c * CT, (c + 1) * CT)
                    s_ps = ps.tile([1, CT], dt.float32, name="s_ps", tag="ps")
                    nc.tensor.matmul(s_ps[:, :], ones_sb[:, 0:1],
                                     exp_g[:, cs], start=True, stop=True)
                    nc.vector.reciprocal(recip[:, cs], s_ps[:, :])
                    r8_ps = ps.tile([E, CT], dt.float32, name="r8_ps",
                                    tag="ps")
                    nc.tensor.matmul(r8_ps[:, :], ones_sb[0:1, 0:E],
                                     recip[:, cs], start=True, stop=True)
                    nc.scalar.copy(rb8[:, cs], r8_ps[:, :])
                    nc.vector.tensor_mul(ew8[:, cs], exp_g[:, cs],
                                         rb8[:, cs])

                # ---- 128-row broadcast of each expert weight ----
                ewb = [act.tile([128, NT], bf, name=f"ewb{e}",
                                tag="ewb", bufs=8) for e in range(E)]
                for e in range(E):
                    for c in range(CH):
                        cs = slice(c * CT, (c + 1) * CT)
                        eb_ps = ps.tile([128, CT], dt.float32,
                                        name="eb_ps", tag="ps")
                        nc.tensor.matmul(
                            eb_ps[:, :], emat_sb[:, e * 128:(e + 1) * 128],
                            ew8[:, cs], start=True, stop=True)
                        nc.scalar.copy(ewb[e][:, cs], eb_ps[:, :])

                # ---- L0 bias+v moving operand: [ew8(8); v*ew_e(24)] ----
                # (DVE has one PSUM read port: stage the row-scatter of
                # [v;1] through SBUF, then mul with the ew-scatter psum.)
                vst = act.tile([32, NT], bf, name="vst", tag="vst")
                bv = act.tile([32, NT], bf, name="bv", tag="bv")
                for c in range(CH):
                    cs = slice(c * CT, (c + 1) * CT)
                    v_ps = ps.tile([32, CT], dt.float32, name="v_ps",
                                   tag="ps")
                    nc.tensor.matmul(v_ps[:, :], vp_sb[:, :],
                                     x0_sb[:, 2, cs], start=True, stop=True)
                    nc.scalar.copy(vst[:, cs], v_ps[:, :])
                    m2_ps = ps.tile([32, CT], dt.float32, name="m2_ps",
                                    tag="ps")
                    nc.tensor.matmul(m2_ps[:, :], em2_sb[:, :],
                                     ew8[:, cs], start=True, stop=True)
                    nc.vector.tensor_mul(bv[:, cs], m2_ps[:, :], vst[:, cs])

                return ew8, ewb, bv, h0

            def body_moe(gate_out):
                ew8, ewb, bv = gate_out[0], gate_out[1], gate_out[2]
                # ---- MoE layers ----
                layers = [
                    (W0_d, None, 4, 4, DH),
                    (W1_d, bias_sb[1], 6, 4, DH),
                    (W2_d, bias_sb[2], 6, 4, DH),
                    (WO_d, bias_sb[3], 6, 2, DM),
                ]
                xcur = None   # list of 4 act tiles (128, NT) for layers >= 1
                y_sb = None

                for li, (wd, b_sb, ktiles, douts, dout_dim) in enumerate(layers):
                    w_tiles = []
                    for e in range(E):
                        wt = wp.tile([128, ktiles, dout_dim], bf,
                                     name=f"w{li}_{e}", tag="w", bufs=3)
                        nc.sync.dma_start(wt, wd[e])
                        w_tiles.append(wt)

                    if li < 3:
                        xnext = [act.tile([128, NT], bf, name=f"x{li + 1}_{m}",
                                          tag="xp", bufs=8) for m in range(4)]
                    else:
                        y_sb = [act.tile([128, NT], bf, name=f"y{m}",
                                         tag="xp", bufs=8) for m in range(2)]

                    def xsrc(kt, cs):
                        if kt < 2:
                            return z_sb[:, kt, cs]
                        if li == 0:
                            return x0_sb[:, kt - 2, cs]
                        return xcur[kt - 2][:, cs]

                    psums = [[ps.tile([128, CT], dt.float32,
                                      name=f"mps{li}_{m}_{c}", tag="ps")
                              for c in range(CH)] for m in range(douts)]
                    for c in range(CH):
                        cs = slice(c * CT, (c + 1) * CT)
                        for m in range(douts):
                            ms = slice(m * 128, (m + 1) * 128)
                            if li == 0:
                                nc.tensor.matmul(
                                    psums[m][c][:, :], bv_w[:, ms],
                                    bv[:, cs], start=True, stop=False)
                            else:
                                nc.tensor.matmul(
                                    psums[m][c][:, :], b_sb[:, ms],
                                    ew8[:, cs], start=True, stop=False)
                    for e in range(E):
                        for kt in range(ktiles):
                            for c in range(CH):
                                cs = slice(c * CT, (c + 1) * CT)
                                xs = xsp.tile([128, CT], bf, name="xs",
                                              tag="xs")
                                nc.vector.tensor_mul(xs[:, :], xsrc(kt, cs),
                                                     ewb[e][:, cs])
                                for m in range(douts):
                                    nc.tensor.matmul(
                                        psums[m][c][:, :],
                                        w_tiles[e][:, kt,
                                                   m * 128:(m + 1) * 128],
                                        xs[:, :],
                                        start=False,
                                        stop=(e == E - 1 and kt == ktiles - 1))
                    for m in range(douts):
                        for c in range(CH):
                            cs = slice(c * CT, (c + 1) * CT)
                            if li < 3:
                                elu(xnext[m][:, cs], psums[m][c][:, :])
                            else:
                                nc.scalar.copy(y_sb[m][:, cs],
                                               psums[m][c][:, :])
                    if li < 3:
                        xcur = xnext

                for m in range(2):
                    nc.sync.dma_start(yT_d[m * 128:(m + 1) * 128, :], y_sb[m])

            HINTS = (mybir.EngineType.PE, mybir.EngineType.DVE,
                     mybir.EngineType.Activation, mybir.EngineType.SP)
            if repeat == 1:
                body_moe(body_gate())
            elif scope == "all":
                with tc.For_i(0, repeat, 1, hint_engines=HINTS):
                    body_moe(body_gate())
            elif scope == "gating":
                with tc.For_i(0, repeat, 1, hint_engines=HINTS):
                    body_gate()
                body_moe(body_gate())
            elif scope == "moe":
                gout = body_gate()
                with tc.For_i(0, repeat, 1, hint_engines=HINTS):
                    body_moe(gout)
            elif scope == "moe1":
                ew8, ewb, bv, h0 = body_gate()
                with tc.For_i(0, repeat, 1, hint_engines=HINTS):
                    # one L1-shaped layer: weights DMA'd, src = h0 tiles
                    wt1 = []
                    for e in range(E):
                        wt = wp.tile([128, 6, DH], bf,
                                     name=f"m1w_{e}", tag="w", bufs=3)
                        nc.sync.dma_start(wt, W1_d[e])
                        wt1.append(wt)
                    xnext1 = [act.tile([128, NT], bf, name=f"m1x_{m}",
                                       tag="xp", bufs=8) for m in range(4)]
                    psums1 = [[ps.tile([128, CT], dt.float32,
                                       name=f"m1ps_{m}_{c}", tag="ps")
                               for c in range(CH)] for m in range(4)]
                    for c in range(CH):
                        cs = slice(c * CT, (c + 1) * CT)
                        for m in range(4):
                            nc.tensor.matmul(
                                psums1[m][c][:, :],
                                bias_sb[1][:, m * 128:(m + 1) * 128],
                                ew8[:, cs], start=True, stop=False)
                    for e in range(E):
                        for kt in range(6):
                            for c in range(CH):
                                cs = slice(c * CT, (c + 1) * CT)
                                src_ap = (z_sb[:, kt, cs] if kt < 2
                                          else h0[kt - 2][:, cs])
                                xs = xsp.tile([128, CT], bf, name="xs",
                                              tag="xs")
                                nc.vector.tensor_mul(xs[:, :], src_ap,
                                                     ewb[e][:, cs])
                                for m in range(4):
                                    nc.tensor.matmul(
                                        psums1[m][c][:, :],
                                        wt1[e][:, kt, m * 128:(m + 1) * 128],
                                        xs[:, :], start=False,
                                        stop=(e == E - 1 and kt == 5))
                    for m in range(4):
                        for c in range(CH):
                            cs = slice(c * CT, (c + 1) * CT)
                            elu(xnext1[m][:, cs], psums1[m][c][:, :])
                body_moe((ew8, ewb, bv))
            else:
                raise ValueError(scope)

    nc.compile()
    return nc


def kernel(z, p_next, v_hip_next, x_curr,
           gw0, gb0, gw1, gb1, gw2, gb2,
           w0, b0, w1, b1, w2, b2, wo, bo):
    if "nc" not in _CACHE:
        _CACHE["nc"] = _build()
    nc = _CACHE["nc"]

    wdict = _prep_weights(
        np.asarray(gw0, np.float32), np.asarray(gb0, np.float32),
        np.asarray(gw1, np.float32), np.asarray(gb1, np.float32),
        np.asarray(gw2, np.float32), np.asarray(gb2, np.float32),
        np.asarray(w0, np.float32), np.asarray(b0, np.float32),
        np.asarray(w1, np.float32), np.asarray(b1, np.float32),
        np.asarray(w2, np.float32), np.asarray(b2, np.float32),
        np.asarray(wo, np.float32), np.asarray(bo, np.float32))

    in_maps = []
    for c in range(NCORES):
        m = _prep_core_inputs(np.asarray(z, np.float32),
                              np.asarray(p_next, np.float32),
                              np.asarray(v_hip_next, np.float32),
                              np.asarray(x_curr, np.float32), c)
        m.update(wdict)
        in_maps.append(m)

    res = bass_utils.run_bass_kernel_spmd(
        nc, in_maps, core_ids=list(range(NCORES)))

    out = np.empty((B, T, DM), np.float32)
    for c in range(NCORES):
        yT = np.asarray(res.results[c]["yT"]).astype(np.float32)  # (DM, NT)
        out[c * BP:(c + 1) * BP] = yT.T.reshape(BP, T, DM)
    return out
